# revision 18
# baseline (speedup 1.0000x reference)
"""Trainium2 Bass kernel for nn_CFGEmbeder (masked attention pooling).

Reference computation (per batch sample, B=128, N=512 nodes, H=512):
    h      = tanh(code_feat @ W_sa + b_sa)         [N, H]
    scores = h @ w_sc (+ b_sc)                      [N]
    attn   = softmax(scores masked by node_mask)    [N]
    out    = tanh(attn @ code_feat)                 [H]

Sharding: pure data parallel over batch; 16 samples per NeuronCore x 8 cores.
b_sc is dropped: softmax is shift invariant, so it cannot affect the output.

Per-core device algorithm (matmuls in fp16 with fp32 PSUM accumulation):
  - x, W_sa, w_sc are cast to fp16 host-side; x loads as 4 group HWDGE DMAs
    on the ACT ring; xT comes from 4 xbar transpose DMAs on the SP ring.
  - mm1: hT[m] = sum_k W[k,m].T @ xT[k]  -> PSUM; tanh+bias fused on ScalarE.
  - scores_s [1, 512] = sum_m w_sc[m-chunk].T @ tanh_hT[m]: M=1 row matmuls
    (1-column weight loads — cheap; the N=1 column formulation costs a full
    128-col LDWEIGHTS per matmul and dominated the first profile).
  - score rows go SBUF -> DRAM scratch -> one gathered [16, 512] load
    (engines cannot move data across partitions; an SBUF->SBUF DMA would
    race the xbar transposes - a known HW deadlock - so bounce via DRAM).
  - one batched masked softmax: masked = (scores + 1000) * mask (the shift
    keeps softmax exact), exp with accum_out giving row sums in one ACT op.
  - attn -> attnT via 4 PE transposes (identity moving operand).
  - pooled_s [1, 512] = sum_c attnT[c,s].T @ x_nat[s,c]: M=1 row matmuls.
  - per-sample fused tanh on ScalarE, row stored straight to the output.
"""

from contextlib import ExitStack

import numpy as np

import concourse.bass as bass
import concourse.bacc as bacc
import concourse.mybir as mybir
import concourse.tile as tile
from concourse.bass_utils import run_bass_kernel_spmd

F16 = mybir.dt.float16
F32 = mybir.dt.float32
I32 = mybir.dt.int32

B, N, H = 128, 512, 512
NCORES = 8
S = B // NCORES          # samples per core
KC = H // 128            # 4 contraction chunks
MC = H // 128            # 4 output-feature chunks
CC = N // 128            # 4 node chunks
GS = 4                   # samples per load/transpose DMA group
NG = S // GS
MASK_SHIFT = 1000.0      # (scores + SHIFT) * mask; softmax is shift invariant


def build_program():
    nc = bacc.Bacc(trn_type="TRN2", target_bir_lowering=False,
                   num_devices=NCORES)

    x_h = nc.dram_tensor("x", [S, N, H], F16, kind="ExternalInput")
    mask_h = nc.dram_tensor("mask", [S, N], I32, kind="ExternalInput")
    wsa_h = nc.dram_tensor("w_sa", [H, H], F16, kind="ExternalInput")
    bsa_h = nc.dram_tensor("b_sa", [H], F32, kind="ExternalInput")
    wsc_h = nc.dram_tensor("w_sc", [H], F16, kind="ExternalInput")
    id_h = nc.dram_tensor("ident", [128, 128], F32, kind="ExternalInput")
    out_h = nc.dram_tensor("out", [S, H], F32, kind="ExternalOutput")
    sc_h = nc.dram_tensor("score_scratch", [S, N], F32)

    x = x_h.ap()
    Tanh = mybir.ActivationFunctionType.Tanh
    Exp = mybir.ActivationFunctionType.Exp
    Alu = mybir.AluOpType

    with tile.TileContext(nc) as tc, ExitStack() as ctx:
        const = ctx.enter_context(tc.tile_pool(name="const", bufs=1))
        xnat_p = ctx.enter_context(tc.tile_pool(name="xnat", bufs=1))
        xt_p = ctx.enter_context(tc.tile_pool(name="xt", bufs=1))
        th_p = ctx.enter_context(tc.tile_pool(name="th", bufs=2))
        row_p = ctx.enter_context(tc.tile_pool(name="row", bufs=3))
        sm_p = ctx.enter_context(tc.tile_pool(name="sm", bufs=1))
        ph_p = ctx.enter_context(tc.tile_pool(name="ph", bufs=5, space="PSUM"))
        pr_p = ctx.enter_context(tc.tile_pool(name="pr", bufs=2, space="PSUM"))
        pa_p = ctx.enter_context(tc.tile_pool(name="pa", bufs=1, space="PSUM"))

        # ---- constants (ACT HWDGE ring; SWDGE would serialize the xbar) ----
        Wf = const.tile([128, KC, H], F16, name="Wf")
        nc.scalar.dma_start(Wf, wsa_h.ap().rearrange("(k p) h -> p k h", p=128))
        wsc = const.tile([128, MC], F16, name="wsc")
        nc.scalar.dma_start(wsc, wsc_h.ap().rearrange("(c p) -> p c", p=128))
        bsa = const.tile([128, MC], F32, name="bsa")
        nc.scalar.dma_start(bsa, bsa_h.ap().rearrange("(c p) -> p c", p=128))
        idf = const.tile([128, 128], F32, name="idf")
        nc.scalar.dma_start(idf, id_h.ap())
        maski = const.tile([S, N], I32, name="maski")
        nc.scalar.dma_start(maski, mask_h.ap())
        maskf = const.tile([S, N], F32, name="maskf")
        nc.vector.tensor_copy(maskf, maski)

        # ---- x: 4 group loads (ACT ring) + 4 xbar transposes (SP ring) ----
        xnat = xnat_p.tile([128, S, CC, H], F16, name="xnat")
        # xt layout: [128(u=feat%128), s, c(node chunk), k(feat chunk), v]
        xt = xt_p.tile([128, S, CC, KC, 128], F16, name="xt")
        for g in range(NG):
            sl = slice(g * GS, (g + 1) * GS)
            nc.scalar.dma_start(
                xnat[:, sl], x[sl].rearrange("s (c p) h -> p s c h", p=128))
            # out[u, (s c k), v] = in[v, (s c)*512 + k*128 + u]
            nc.sync.dma_start(xt[:, sl], xnat[:, sl], transpose=True)

        # ---- phase A: per-sample matmul1 + tanh + scores row ----
        for s in range(S):
            th = th_p.tile([128, MC, N], F16, name="th")
            for m in range(MC):
                ph = ph_p.tile([128, N], F32, name="ph")
                for k in range(KC):
                    nc.tensor.matmul(
                        ph,
                        lhsT=Wf[:, k, m * 128:(m + 1) * 128],
                        rhs=xt[:, s, :, k, :],
                        start=(k == 0),
                        stop=(k == KC - 1),
                    )
                nc.scalar.activation(th[:, m, :], ph, Tanh,
                                     bias=bsa[:, m:m + 1])

            psr = pr_p.tile([1, N], F32, name="prow")
            for m in range(MC):
                nc.tensor.matmul(
                    psr,
                    lhsT=wsc[:, m:m + 1],
                    rhs=th[:, m, :],
                    start=(m == 0),
                    stop=(m == MC - 1),
                )
            srow = row_p.tile([1, N], F32, name="srow")
            nc.vector.tensor_copy(srow, psr)
            nc.scalar.dma_start(sc_h.ap()[s:s + 1], srow)

        # ---- phase B: softmax over nodes for all samples at once ----
        scores = sm_p.tile([S, N], F32, name="scores")
        nc.scalar.dma_start(scores, sc_h.ap())

        masked = sm_p.tile([S, N], F32, name="masked")
        nc.vector.scalar_tensor_tensor(masked, scores, MASK_SHIFT, maskf,
                                       op0=Alu.add, op1=Alu.mult)
        nmax = sm_p.tile([S, 1], F32, name="nmax")
        nc.vector.tensor_reduce(nmax, masked, axis=mybir.AxisListType.X,
                                op=Alu.max, negate=True)
        ex = sm_p.tile([S, N], F32, name="ex")
        esum = sm_p.tile([S, 1], F32, name="esum")
        nc.scalar.activation(ex, masked, Exp, bias=nmax, accum_out=esum)
        rinv = sm_p.tile([S, 1], F32, name="rinv")
        nc.vector.reciprocal(rinv, esum)
        attn = sm_p.tile([S, N], F32, name="attn")
        nc.vector.tensor_scalar_mul(attn, ex, rinv)

        psum_aT = pa_p.tile([128, CC * S], F32, name="paT")
        for c in range(CC):
            nc.tensor.transpose(psum_aT[:, c * S:(c + 1) * S],
                                attn[:, c * 128:(c + 1) * 128],
                                idf[0:S, 0:S])
        attnT = sm_p.tile([128, CC * S], F16, name="attnT")
        nc.vector.tensor_copy(attnT, psum_aT)

        # ---- phase C: attention pooling ----
        for s in range(S):
            ppr = pr_p.tile([1, H], F32, name="prow")
            for c in range(CC):
                nc.tensor.matmul(
                    ppr,
                    lhsT=attnT[:, c * S + s:c * S + s + 1],
                    rhs=xnat[:, s, c, :],
                    start=(c == 0),
                    stop=(c == CC - 1),
                )
            orow = row_p.tile([1, H], F32, name="orow")
            nc.scalar.activation(orow, ppr, Tanh)
            nc.scalar.dma_start(out_h.ap()[s:s + 1], orow)

    nc.finalize()
    return nc


_CACHE = {}


def _get_nc():
    if "nc" not in _CACHE:
        _CACHE["nc"] = build_program()
    return _CACHE["nc"]


def make_in_maps(code_feat, node_mask, W_sa, b_sa, w_sc):
    ident = np.eye(128, dtype=np.float32)
    x16 = np.asarray(code_feat, dtype=np.float16)
    w16 = np.asarray(W_sa, dtype=np.float16)
    wsc16 = np.asarray(w_sc, dtype=np.float16)
    in_maps = []
    for i in range(NCORES):
        sl = slice(i * S, (i + 1) * S)
        in_maps.append({
            "x": np.ascontiguousarray(x16[sl]),
            "mask": np.ascontiguousarray(node_mask[sl], dtype=np.int32),
            "w_sa": w16,
            "b_sa": np.asarray(b_sa, dtype=np.float32),
            "w_sc": wsc16,
            "ident": ident,
        })
    return in_maps


def kernel(code_feat, node_mask, W_sa, b_sa, w_sc, b_sc=None, **_ignored):
    code_feat = np.asarray(code_feat)
    node_mask = np.asarray(node_mask)
    nc = _get_nc()
    in_maps = make_in_maps(code_feat, node_mask, W_sa, b_sa, w_sc)
    res = run_bass_kernel_spmd(nc, in_maps, list(range(NCORES)))
    out = np.concatenate([r["out"] for r in res.results], axis=0)
    return out.astype(np.float32)


# revision 19
# speedup vs baseline: 1.0454x; 1.0454x over previous
"""Trainium2 Bass kernel for nn_CFGEmbeder (masked attention pooling).

Reference computation (per batch sample, B=128, N=512 nodes, H=512):
    h      = tanh(code_feat @ W_sa + b_sa)         [N, H]
    scores = h @ w_sc (+ b_sc)                      [N]
    attn   = softmax(scores masked by node_mask)    [N]
    out    = tanh(attn @ code_feat)                 [H]

Sharding: pure data parallel over batch; 16 samples per NeuronCore x 8 cores.
b_sc is dropped: softmax is shift invariant, so it cannot affect the output.

Per-core device algorithm (matmuls in fp16 with fp32 PSUM accumulation):
  - x, W_sa, w_sc are cast to fp16 host-side; x loads as 4 group HWDGE DMAs
    on the ACT ring; xT comes from 4 xbar transpose DMAs on the SP ring.
  - mm1: hT[m] = sum_k W[k,m].T @ xT[k]  -> PSUM; tanh+bias fused on ScalarE.
  - scores_s [1, 512] = sum_m w_sc[m-chunk].T @ tanh_hT[m]: M=1 row matmuls
    (1-column weight loads — cheap; the N=1 column formulation costs a full
    128-col LDWEIGHTS per matmul and dominated the first profile).
  - score rows go SBUF -> DRAM scratch -> one gathered [16, 512] load
    (engines cannot move data across partitions; an SBUF->SBUF DMA would
    race the xbar transposes - a known HW deadlock - so bounce via DRAM).
  - one batched masked softmax: masked = (scores + 1000) * mask (the shift
    keeps softmax exact), exp with accum_out giving row sums in one ACT op.
  - attn -> attnT via 4 PE transposes (identity moving operand).
  - pooled_s [1, 512] = sum_c attnT[c,s].T @ x_nat[s,c]: M=1 row matmuls.
  - per-sample fused tanh on ScalarE, row stored straight to the output.
"""

from contextlib import ExitStack

import numpy as np

import concourse.bass as bass
import concourse.bacc as bacc
import concourse.mybir as mybir
import concourse.tile as tile
from concourse.bass_utils import run_bass_kernel_spmd

F16 = mybir.dt.float16
F32 = mybir.dt.float32
I32 = mybir.dt.int32

B, N, H = 128, 512, 512
NCORES = 8
S = B // NCORES          # samples per core
KC = H // 128            # 4 contraction chunks
MC = H // 128            # 4 output-feature chunks
CC = N // 128            # 4 node chunks
GS = 4                   # samples per load/transpose DMA group
NG = S // GS
MASK_SHIFT = 1000.0      # (scores + SHIFT) * mask; softmax is shift invariant


def build_program():
    nc = bacc.Bacc(trn_type="TRN2", target_bir_lowering=False,
                   num_devices=NCORES)

    x_h = nc.dram_tensor("x", [S, N, H], F16, kind="ExternalInput")
    mask_h = nc.dram_tensor("mask", [S, N], I32, kind="ExternalInput")
    wsa_h = nc.dram_tensor("w_sa", [H, H], F16, kind="ExternalInput")
    bsa_h = nc.dram_tensor("b_sa", [H], F32, kind="ExternalInput")
    wsc_h = nc.dram_tensor("w_sc", [H], F16, kind="ExternalInput")
    id_h = nc.dram_tensor("ident", [128, 128], F32, kind="ExternalInput")
    out_h = nc.dram_tensor("out", [S, H], F32, kind="ExternalOutput")
    sc_h = nc.dram_tensor("score_scratch", [S, N], F32)

    x = x_h.ap()
    Tanh = mybir.ActivationFunctionType.Tanh
    Exp = mybir.ActivationFunctionType.Exp
    Alu = mybir.AluOpType

    with tile.TileContext(nc) as tc, ExitStack() as ctx:
        const = ctx.enter_context(tc.tile_pool(name="const", bufs=1))
        xnat_p = ctx.enter_context(tc.tile_pool(name="xnat", bufs=1))
        xt_p = ctx.enter_context(tc.tile_pool(name="xt", bufs=1))
        th_p = ctx.enter_context(tc.tile_pool(name="th", bufs=2))
        row_p = ctx.enter_context(tc.tile_pool(name="row", bufs=3))
        sm_p = ctx.enter_context(tc.tile_pool(name="sm", bufs=1))
        ph_p = ctx.enter_context(tc.tile_pool(name="ph", bufs=5, space="PSUM"))
        pr_p = ctx.enter_context(tc.tile_pool(name="pr", bufs=2, space="PSUM"))
        pa_p = ctx.enter_context(tc.tile_pool(name="pa", bufs=1, space="PSUM"))

        # ---- constants (ACT HWDGE ring; SWDGE would serialize the xbar) ----
        Wf = const.tile([128, KC, H], F16, name="Wf")
        nc.scalar.dma_start(Wf, wsa_h.ap().rearrange("(k p) h -> p k h", p=128))
        wsc = const.tile([128, MC], F16, name="wsc")
        nc.scalar.dma_start(wsc, wsc_h.ap().rearrange("(c p) -> p c", p=128))
        bsa = const.tile([128, MC], F32, name="bsa")
        nc.scalar.dma_start(bsa, bsa_h.ap().rearrange("(c p) -> p c", p=128))
        idf = const.tile([128, 128], F32, name="idf")
        nc.scalar.dma_start(idf, id_h.ap())
        maski = const.tile([S, N], I32, name="maski")
        nc.scalar.dma_start(maski, mask_h.ap())
        maskf = const.tile([S, N], F32, name="maskf")
        nc.vector.tensor_copy(maskf, maski)

        # ---- x: 4 group loads (ACT ring) + 4 xbar transposes (SP ring) ----
        xnat = xnat_p.tile([128, S, CC, H], F16, name="xnat")
        # xt layout: [128(u=feat%128), s, c(node chunk), k(feat chunk), v]
        xt = xt_p.tile([128, S, CC, KC, 128], F16, name="xt")
        for g in range(NG):
            sl = slice(g * GS, (g + 1) * GS)
            nc.scalar.dma_start(
                xnat[:, sl], x[sl].rearrange("s (c p) h -> p s c h", p=128))
            # out[u, (s c k), v] = in[v, (s c)*512 + k*128 + u]
            nc.sync.dma_start(xt[:, sl], xnat[:, sl], transpose=True)

        # ---- phase A: per-sample matmul1 + tanh + scores row ----
        for s in range(S):
            th = th_p.tile([128, MC, N], F16, name="th")
            for m in range(MC):
                ph = ph_p.tile([128, N], F32, name="ph")
                for k in range(KC):
                    nc.tensor.matmul(
                        ph,
                        lhsT=Wf[:, k, m * 128:(m + 1) * 128],
                        rhs=xt[:, s, :, k, :],
                        start=(k == 0),
                        stop=(k == KC - 1),
                    )
                nc.scalar.activation(th[:, m, :], ph, Tanh,
                                     bias=bsa[:, m:m + 1])

            psr = pr_p.tile([1, N], F32, name="prow")
            for m in range(MC):
                nc.tensor.matmul(
                    psr,
                    lhsT=wsc[:, m:m + 1],
                    rhs=th[:, m, :],
                    start=(m == 0),
                    stop=(m == MC - 1),
                )
            srow = row_p.tile([1, N], F32, name="srow")
            nc.vector.tensor_copy(srow, psr)
            nc.gpsimd.dma_start(sc_h.ap()[s:s + 1], srow)

        # ---- phase B: softmax over nodes for all samples at once ----
        scores = sm_p.tile([S, N], F32, name="scores")
        nc.gpsimd.dma_start(scores, sc_h.ap())

        masked = sm_p.tile([S, N], F32, name="masked")
        nc.vector.scalar_tensor_tensor(masked, scores, MASK_SHIFT, maskf,
                                       op0=Alu.add, op1=Alu.mult)
        nmax = sm_p.tile([S, 1], F32, name="nmax")
        nc.vector.tensor_reduce(nmax, masked, axis=mybir.AxisListType.X,
                                op=Alu.max, negate=True)
        ex = sm_p.tile([S, N], F32, name="ex")
        esum = sm_p.tile([S, 1], F32, name="esum")
        nc.scalar.activation(ex, masked, Exp, bias=nmax, accum_out=esum)
        rinv = sm_p.tile([S, 1], F32, name="rinv")
        nc.vector.reciprocal(rinv, esum)
        attn = sm_p.tile([S, N], F32, name="attn")
        nc.vector.tensor_scalar_mul(attn, ex, rinv)

        psum_aT = pa_p.tile([128, CC * S], F32, name="paT")
        for c in range(CC):
            nc.tensor.transpose(psum_aT[:, c * S:(c + 1) * S],
                                attn[:, c * 128:(c + 1) * 128],
                                idf[0:S, 0:S])
        attnT = sm_p.tile([128, CC * S], F16, name="attnT")
        nc.vector.tensor_copy(attnT, psum_aT)

        # ---- phase C: attention pooling ----
        for s in range(S):
            ppr = pr_p.tile([1, H], F32, name="prow")
            for c in range(CC):
                nc.tensor.matmul(
                    ppr,
                    lhsT=attnT[:, c * S + s:c * S + s + 1],
                    rhs=xnat[:, s, c, :],
                    start=(c == 0),
                    stop=(c == CC - 1),
                )
            orow = row_p.tile([1, H], F32, name="orow")
            nc.scalar.activation(orow, ppr, Tanh)
            nc.gpsimd.dma_start(out_h.ap()[s:s + 1], orow)

    nc.finalize()
    return nc


_CACHE = {}


def _get_nc():
    if "nc" not in _CACHE:
        _CACHE["nc"] = build_program()
    return _CACHE["nc"]


def make_in_maps(code_feat, node_mask, W_sa, b_sa, w_sc):
    ident = np.eye(128, dtype=np.float32)
    x16 = np.asarray(code_feat, dtype=np.float16)
    w16 = np.asarray(W_sa, dtype=np.float16)
    wsc16 = np.asarray(w_sc, dtype=np.float16)
    in_maps = []
    for i in range(NCORES):
        sl = slice(i * S, (i + 1) * S)
        in_maps.append({
            "x": np.ascontiguousarray(x16[sl]),
            "mask": np.ascontiguousarray(node_mask[sl], dtype=np.int32),
            "w_sa": w16,
            "b_sa": np.asarray(b_sa, dtype=np.float32),
            "w_sc": wsc16,
            "ident": ident,
        })
    return in_maps


def kernel(code_feat, node_mask, W_sa, b_sa, w_sc, b_sc=None, **_ignored):
    code_feat = np.asarray(code_feat)
    node_mask = np.asarray(node_mask)
    nc = _get_nc()
    in_maps = make_in_maps(code_feat, node_mask, W_sa, b_sa, w_sc)
    res = run_bass_kernel_spmd(nc, in_maps, list(range(NCORES)))
    out = np.concatenate([r["out"] for r in res.results], axis=0)
    return out.astype(np.float32)


# revision 20
# speedup vs baseline: 1.1810x; 1.1297x over previous
"""Trainium2 Bass kernel for nn_CFGEmbeder (masked attention pooling).

Reference computation (per batch sample, B=128, N=512 nodes, H=512):
    h      = tanh(code_feat @ W_sa + b_sa)         [N, H]
    scores = h @ w_sc (+ b_sc)                      [N]
    attn   = softmax(scores masked by node_mask)    [N]
    out    = tanh(attn @ code_feat)                 [H]

Sharding: pure data parallel over batch; 16 samples per NeuronCore x 8 cores.
b_sc is dropped: softmax is shift invariant, so it cannot affect the output.

Per-core device algorithm (matmuls in fp16 with fp32 PSUM accumulation):
  - x, W_sa, w_sc are cast to fp16 host-side; the host also supplies x
    pre-transposed (xT, partition=feature) so no on-device transpose is
    needed. xT loads ride the otherwise-idle SWDGE queue (feeds matmul1
    immediately); x-natural loads ride the ACT HWDGE ring (only needed by
    the late pooling phase, so ScalarE's busy stream can delay them freely).
  - mm1: hT[m] = sum_k W[k,m].T @ xT[k]  -> PSUM; tanh+bias fused on ScalarE.
  - scores_s [1, 512] = sum_m w_sc[m-chunk].T @ tanh_hT[m]: M=1 row matmuls
    (1-column weight loads — cheap; the N=1 column formulation costs a full
    128-col LDWEIGHTS per matmul and dominated the first profile).
  - score rows go SBUF -> DRAM scratch -> one gathered [16, 512] load
    (engines cannot move data across partitions; an SBUF->SBUF DMA would
    race the xbar transposes - a known HW deadlock - so bounce via DRAM).
  - one batched masked softmax: masked = (scores + 1000) * mask (the shift
    keeps softmax exact), exp with accum_out giving row sums in one ACT op.
  - attn -> attnT via 4 PE transposes (identity moving operand).
  - pooled_s [1, 512] = sum_c attnT[c,s].T @ x_nat[s,c]: M=1 row matmuls.
  - per-sample fused tanh on ScalarE, row stored straight to the output.
"""

from contextlib import ExitStack

import numpy as np

import concourse.bass as bass
import concourse.bacc as bacc
import concourse.mybir as mybir
import concourse.tile as tile
from concourse.bass_utils import run_bass_kernel_spmd

F16 = mybir.dt.float16
F32 = mybir.dt.float32
I32 = mybir.dt.int32

B, N, H = 128, 512, 512
NCORES = 8
S = B // NCORES          # samples per core
KC = H // 128            # 4 contraction chunks
MC = H // 128            # 4 output-feature chunks
CC = N // 128            # 4 node chunks
GS = 4                   # samples per load/transpose DMA group
NG = S // GS
MASK_SHIFT = 1000.0      # (scores + SHIFT) * mask; softmax is shift invariant


def build_program():
    nc = bacc.Bacc(trn_type="TRN2", target_bir_lowering=False,
                   num_devices=NCORES)

    x_h = nc.dram_tensor("x", [S, N, H], F16, kind="ExternalInput")
    xt_h = nc.dram_tensor("xT", [S, H, N], F16, kind="ExternalInput")
    mask_h = nc.dram_tensor("mask", [S, N], I32, kind="ExternalInput")
    wsa_h = nc.dram_tensor("w_sa", [H, H], F16, kind="ExternalInput")
    bsa_h = nc.dram_tensor("b_sa", [H], F32, kind="ExternalInput")
    wsc_h = nc.dram_tensor("w_sc", [H], F16, kind="ExternalInput")
    id_h = nc.dram_tensor("ident", [128, 128], F32, kind="ExternalInput")
    out_h = nc.dram_tensor("out", [S, H], F32, kind="ExternalOutput")
    sc_h = nc.dram_tensor("score_scratch", [S, N], F32)

    x = x_h.ap()
    Tanh = mybir.ActivationFunctionType.Tanh
    Exp = mybir.ActivationFunctionType.Exp
    Alu = mybir.AluOpType

    with tile.TileContext(nc) as tc, ExitStack() as ctx:
        const = ctx.enter_context(tc.tile_pool(name="const", bufs=1))
        xnat_p = ctx.enter_context(tc.tile_pool(name="xnat", bufs=1))
        xt_p = ctx.enter_context(tc.tile_pool(name="xt", bufs=1))
        th_p = ctx.enter_context(tc.tile_pool(name="th", bufs=2))
        row_p = ctx.enter_context(tc.tile_pool(name="row", bufs=3))
        sm_p = ctx.enter_context(tc.tile_pool(name="sm", bufs=1))
        ph_p = ctx.enter_context(tc.tile_pool(name="ph", bufs=5, space="PSUM"))
        pr_p = ctx.enter_context(tc.tile_pool(name="pr", bufs=2, space="PSUM"))
        pa_p = ctx.enter_context(tc.tile_pool(name="pa", bufs=1, space="PSUM"))

        # ---- constants (ACT HWDGE ring; SWDGE would serialize the xbar) ----
        Wf = const.tile([128, KC, H], F16, name="Wf")
        nc.scalar.dma_start(Wf, wsa_h.ap().rearrange("(k p) h -> p k h", p=128))
        wsc = const.tile([128, MC], F16, name="wsc")
        nc.scalar.dma_start(wsc, wsc_h.ap().rearrange("(c p) -> p c", p=128))
        bsa = const.tile([128, MC], F32, name="bsa")
        nc.scalar.dma_start(bsa, bsa_h.ap().rearrange("(c p) -> p c", p=128))
        idf = const.tile([128, 128], F32, name="idf")
        nc.scalar.dma_start(idf, id_h.ap())
        maski = const.tile([S, N], I32, name="maski")
        nc.scalar.dma_start(maski, mask_h.ap())
        maskf = const.tile([S, N], F32, name="maskf")
        nc.vector.tensor_copy(maskf, maski)

        # ---- x loads: xT on SWDGE (early, feeds mm1), x-nat on ACT ring ----
        xnat = xnat_p.tile([128, S, CC, H], F16, name="xnat")
        # xt layout: [128(u=feat%128), s, k(feat chunk), n(node)]
        xt = xt_p.tile([128, S, KC, N], F16, name="xt")
        xth = xt_h.ap()
        for g in range(NG):
            sl = slice(g * GS, (g + 1) * GS)
            nc.gpsimd.dma_start(
                xt[:, sl], xth[sl].rearrange("s (k p) n -> p s k n", p=128))
            nc.scalar.dma_start(
                xnat[:, sl], x[sl].rearrange("s (c p) h -> p s c h", p=128))

        # ---- phase A: per-sample matmul1 + tanh + scores row ----
        for s in range(S):
            th = th_p.tile([128, MC, N], F16, name="th")
            for m in range(MC):
                ph = ph_p.tile([128, N], F32, name="ph")
                for k in range(KC):
                    nc.tensor.matmul(
                        ph,
                        lhsT=Wf[:, k, m * 128:(m + 1) * 128],
                        rhs=xt[:, s, k, :],
                        start=(k == 0),
                        stop=(k == KC - 1),
                    )
                nc.scalar.activation(th[:, m, :], ph, Tanh,
                                     bias=bsa[:, m:m + 1])

            psr = pr_p.tile([1, N], F32, name="prow")
            for m in range(MC):
                nc.tensor.matmul(
                    psr,
                    lhsT=wsc[:, m:m + 1],
                    rhs=th[:, m, :],
                    start=(m == 0),
                    stop=(m == MC - 1),
                )
            srow = row_p.tile([1, N], F32, name="srow")
            nc.vector.tensor_copy(srow, psr)
            nc.gpsimd.dma_start(sc_h.ap()[s:s + 1], srow)

        # ---- phase B: softmax over nodes for all samples at once ----
        scores = sm_p.tile([S, N], F32, name="scores")
        nc.gpsimd.dma_start(scores, sc_h.ap())

        masked = sm_p.tile([S, N], F32, name="masked")
        nc.vector.scalar_tensor_tensor(masked, scores, MASK_SHIFT, maskf,
                                       op0=Alu.add, op1=Alu.mult)
        nmax = sm_p.tile([S, 1], F32, name="nmax")
        nc.vector.tensor_reduce(nmax, masked, axis=mybir.AxisListType.X,
                                op=Alu.max, negate=True)
        ex = sm_p.tile([S, N], F32, name="ex")
        esum = sm_p.tile([S, 1], F32, name="esum")
        nc.scalar.activation(ex, masked, Exp, bias=nmax, accum_out=esum)
        rinv = sm_p.tile([S, 1], F32, name="rinv")
        nc.vector.reciprocal(rinv, esum)
        attn = sm_p.tile([S, N], F32, name="attn")
        nc.vector.tensor_scalar_mul(attn, ex, rinv)

        psum_aT = pa_p.tile([128, CC * S], F32, name="paT")
        for c in range(CC):
            nc.tensor.transpose(psum_aT[:, c * S:(c + 1) * S],
                                attn[:, c * 128:(c + 1) * 128],
                                idf[0:S, 0:S])
        attnT = sm_p.tile([128, CC * S], F16, name="attnT")
        nc.vector.tensor_copy(attnT, psum_aT)

        # ---- phase C: attention pooling ----
        for s in range(S):
            ppr = pr_p.tile([1, H], F32, name="prow")
            for c in range(CC):
                nc.tensor.matmul(
                    ppr,
                    lhsT=attnT[:, c * S + s:c * S + s + 1],
                    rhs=xnat[:, s, c, :],
                    start=(c == 0),
                    stop=(c == CC - 1),
                )
            orow = row_p.tile([1, H], F32, name="orow")
            nc.scalar.activation(orow, ppr, Tanh)
            nc.gpsimd.dma_start(out_h.ap()[s:s + 1], orow)

    nc.finalize()
    return nc


_CACHE = {}


def _get_nc():
    if "nc" not in _CACHE:
        _CACHE["nc"] = build_program()
    return _CACHE["nc"]


def make_in_maps(code_feat, node_mask, W_sa, b_sa, w_sc):
    ident = np.eye(128, dtype=np.float32)
    x16 = np.asarray(code_feat, dtype=np.float16)
    w16 = np.asarray(W_sa, dtype=np.float16)
    wsc16 = np.asarray(w_sc, dtype=np.float16)
    in_maps = []
    for i in range(NCORES):
        sl = slice(i * S, (i + 1) * S)
        in_maps.append({
            "x": np.ascontiguousarray(x16[sl]),
            "xT": np.ascontiguousarray(x16[sl].transpose(0, 2, 1)),
            "mask": np.ascontiguousarray(node_mask[sl], dtype=np.int32),
            "w_sa": w16,
            "b_sa": np.asarray(b_sa, dtype=np.float32),
            "w_sc": wsc16,
            "ident": ident,
        })
    return in_maps


def kernel(code_feat, node_mask, W_sa, b_sa, w_sc, b_sc=None, **_ignored):
    code_feat = np.asarray(code_feat)
    node_mask = np.asarray(node_mask)
    nc = _get_nc()
    in_maps = make_in_maps(code_feat, node_mask, W_sa, b_sa, w_sc)
    res = run_bass_kernel_spmd(nc, in_maps, list(range(NCORES)))
    out = np.concatenate([r["out"] for r in res.results], axis=0)
    return out.astype(np.float32)


# revision 21
# speedup vs baseline: 1.4912x; 1.2627x over previous
"""Trainium2 Bass kernel for nn_CFGEmbeder (masked attention pooling).

Reference computation (per batch sample, B=128, N=512 nodes, H=512):
    h      = tanh(code_feat @ W_sa + b_sa)         [N, H]
    scores = h @ w_sc (+ b_sc)                      [N]
    attn   = softmax(scores masked by node_mask)    [N]
    out    = tanh(attn @ code_feat)                 [H]

Sharding: pure data parallel over batch; 16 samples per NeuronCore x 8 cores.
b_sc is dropped: softmax is shift invariant, so it cannot affect the output.

Per-core device algorithm (matmuls in fp16 with fp32 PSUM accumulation):
  - x, W_sa, w_sc are cast to fp16 host-side; the host also supplies x
    pre-transposed (xT, partition=feature) so no on-device transpose is
    needed. xT loads ride the otherwise-idle SWDGE queue (feeds matmul1
    immediately); x-natural loads ride the ACT HWDGE ring (only needed by
    the late pooling phase, so ScalarE's busy stream can delay them freely).
  - mm1: hT[m] = sum_k W[k,m].T @ xT[k]  -> PSUM; tanh+bias fused on ScalarE.
  - scores_s [1, 512] = sum_m w_sc[m-chunk].T @ tanh_hT[m]: M=1 row matmuls
    (1-column weight loads — cheap; the N=1 column formulation costs a full
    128-col LDWEIGHTS per matmul and dominated the first profile).
  - score rows go SBUF -> DRAM scratch -> one gathered [16, 512] load
    (engines cannot move data across partitions; an SBUF->SBUF DMA would
    race the xbar transposes - a known HW deadlock - so bounce via DRAM).
  - one batched masked softmax: masked = (scores + 1000) * mask (the shift
    keeps softmax exact), exp with accum_out giving row sums in one ACT op.
  - attn -> attnT via 4 PE transposes (identity moving operand).
  - pooled_s [1, 512] = sum_c attnT[c,s].T @ x_nat[s,c]: M=1 row matmuls.
  - per-sample fused tanh on ScalarE, row stored straight to the output.
"""

from contextlib import ExitStack

import numpy as np

import concourse.bass as bass
import concourse.bacc as bacc
import concourse.mybir as mybir
import concourse.tile as tile
from concourse.bass_utils import run_bass_kernel_spmd

F16 = mybir.dt.float16
F32 = mybir.dt.float32
I32 = mybir.dt.int32

B, N, H = 128, 512, 512
NCORES = 8
S = B // NCORES          # samples per core
KC = H // 128            # 4 contraction chunks
MC = H // 128            # 4 output-feature chunks
CC = N // 128            # 4 node chunks
GS = 4                   # samples per load/transpose DMA group
NG = S // GS
MASK_SHIFT = 1000.0      # (scores + SHIFT) * mask; softmax is shift invariant


def build_program():
    nc = bacc.Bacc(trn_type="TRN2", target_bir_lowering=False,
                   num_devices=NCORES)

    x_h = nc.dram_tensor("x", [S, N, H], F16, kind="ExternalInput")
    xt_h = nc.dram_tensor("xT", [S, H, N], F16, kind="ExternalInput")
    mask_h = nc.dram_tensor("mask", [S, N], I32, kind="ExternalInput")
    wsa_h = nc.dram_tensor("w_sa", [H, H], F16, kind="ExternalInput")
    bsa_h = nc.dram_tensor("b_sa", [H], F32, kind="ExternalInput")
    wsc_h = nc.dram_tensor("w_sc", [H], F16, kind="ExternalInput")
    id_h = nc.dram_tensor("ident", [128, 128], F32, kind="ExternalInput")
    out_h = nc.dram_tensor("out", [S, H], F32, kind="ExternalOutput")
    sc_h = nc.dram_tensor("score_scratch", [S, N], F32)

    x = x_h.ap()
    Tanh = mybir.ActivationFunctionType.Tanh
    Exp = mybir.ActivationFunctionType.Exp
    Alu = mybir.AluOpType

    with tile.TileContext(nc) as tc, ExitStack() as ctx:
        const = ctx.enter_context(tc.tile_pool(name="const", bufs=1))
        xnat_p = ctx.enter_context(tc.tile_pool(name="xnat", bufs=1))
        xt_p = ctx.enter_context(tc.tile_pool(name="xt", bufs=1))
        th_p = ctx.enter_context(tc.tile_pool(name="th", bufs=2))
        row_p = ctx.enter_context(tc.tile_pool(name="row", bufs=3))
        sm_p = ctx.enter_context(tc.tile_pool(name="sm", bufs=1))
        ph_p = ctx.enter_context(tc.tile_pool(name="ph", bufs=5, space="PSUM"))
        pr_p = ctx.enter_context(tc.tile_pool(name="pr", bufs=2, space="PSUM"))
        pa_p = ctx.enter_context(tc.tile_pool(name="pa", bufs=1, space="PSUM"))

        # ---- constants (ACT HWDGE ring; SWDGE would serialize the xbar) ----
        Wf = const.tile([128, KC, H], F16, name="Wf")
        nc.scalar.dma_start(Wf, wsa_h.ap().rearrange("(k p) h -> p k h", p=128))
        wsc = const.tile([128, MC], F16, name="wsc")
        nc.scalar.dma_start(wsc, wsc_h.ap().rearrange("(c p) -> p c", p=128))
        bsa = const.tile([128, MC], F32, name="bsa")
        nc.scalar.dma_start(bsa, bsa_h.ap().rearrange("(c p) -> p c", p=128))
        idf = const.tile([128, 128], F32, name="idf")
        nc.scalar.dma_start(idf, id_h.ap())
        maski = const.tile([S, N], I32, name="maski")
        nc.scalar.dma_start(maski, mask_h.ap())
        maskf = const.tile([S, N], F32, name="maskf")
        nc.vector.tensor_copy(maskf, maski)

        # ---- x loads: all on the SWDGE queue (gpsimd issues nothing else,
        # so DMA issues never block a compute engine's instruction stream;
        # an HWDGE ring issue stalls its engine until the ring drains).
        # xT groups first (feed matmul1 right away), x-natural after (only
        # the late pooling phase needs it).
        xnat = xnat_p.tile([128, S, CC, H], F16, name="xnat")
        # xt layout: [128(u=feat%128), s, k(feat chunk), n(node)]
        xt = xt_p.tile([128, S, KC, N], F16, name="xt")
        xth = xt_h.ap()
        for g in range(NG):
            sl = slice(g * GS, (g + 1) * GS)
            nc.gpsimd.dma_start(
                xt[:, sl], xth[sl].rearrange("s (k p) n -> p s k n", p=128))
        for g in range(NG):
            sl = slice(g * GS, (g + 1) * GS)
            nc.gpsimd.dma_start(
                xnat[:, sl], x[sl].rearrange("s (c p) h -> p s c h", p=128))

        # ---- phase A: paired-sample matmul1 + tanh + scores rows ----
        # Samples are processed in pairs sharing each W stationary load:
        # LDWEIGHTS(k,m) then two matmuls streaming both samples' xT.
        for s0 in range(0, S, 2):
            ths = [th_p.tile([128, MC, N], F16, name=f"th{i}") for i in (0, 1)]
            for m in range(MC):
                phs = [ph_p.tile([128, N], F32, name="ph") for _ in (0, 1)]
                for k in range(KC):
                    for i in (0, 1):
                        nc.tensor.matmul(
                            phs[i],
                            lhsT=Wf[:, k, m * 128:(m + 1) * 128],
                            rhs=xt[:, s0 + i, k, :],
                            start=(k == 0),
                            stop=(k == KC - 1),
                        )
                for i in (0, 1):
                    nc.scalar.activation(ths[i][:, m, :], phs[i], Tanh,
                                         bias=bsa[:, m:m + 1])

            for i in (0, 1):
                psr = pr_p.tile([1, N], F32, name="prow")
                for m in range(MC):
                    nc.tensor.matmul(
                        psr,
                        lhsT=wsc[:, m:m + 1],
                        rhs=ths[i][:, m, :],
                        start=(m == 0),
                        stop=(m == MC - 1),
                    )
                srow = row_p.tile([1, N], F32, name="srow")
                nc.vector.tensor_copy(srow, psr)
                nc.gpsimd.dma_start(sc_h.ap()[s0 + i:s0 + i + 1], srow)

        # ---- phase B: softmax over nodes for all samples at once ----
        scores = sm_p.tile([S, N], F32, name="scores")
        nc.gpsimd.dma_start(scores, sc_h.ap())

        masked = sm_p.tile([S, N], F32, name="masked")
        nc.vector.scalar_tensor_tensor(masked, scores, MASK_SHIFT, maskf,
                                       op0=Alu.add, op1=Alu.mult)
        nmax = sm_p.tile([S, 1], F32, name="nmax")
        nc.vector.tensor_reduce(nmax, masked, axis=mybir.AxisListType.X,
                                op=Alu.max, negate=True)
        ex = sm_p.tile([S, N], F32, name="ex")
        esum = sm_p.tile([S, 1], F32, name="esum")
        nc.scalar.activation(ex, masked, Exp, bias=nmax, accum_out=esum)
        rinv = sm_p.tile([S, 1], F32, name="rinv")
        nc.vector.reciprocal(rinv, esum)
        attn = sm_p.tile([S, N], F32, name="attn")
        nc.vector.tensor_scalar_mul(attn, ex, rinv)

        # Keep the PE HAM window busy through the softmax bubble so the
        # pooling matmuls run at 2.4 GHz instead of re-warming from 1.2.
        phw = ph_p.tile([128, N], F32, name="ph")
        for w in range(20):
            nc.tensor.matmul(phw, lhsT=Wf[:, 0, 0:128], rhs=xt[:, S - 1, 0, :],
                             start=(w == 0), stop=(w == 19))

        psum_aT = pa_p.tile([128, CC * S], F32, name="paT")
        for c in range(CC):
            nc.tensor.transpose(psum_aT[:, c * S:(c + 1) * S],
                                attn[:, c * 128:(c + 1) * 128],
                                idf[0:S, 0:S])
        attnT = sm_p.tile([128, CC * S], F16, name="attnT")
        nc.vector.tensor_copy(attnT, psum_aT)

        # ---- phase C: attention pooling ----
        for s in range(S):
            ppr = pr_p.tile([1, H], F32, name="prow")
            for c in range(CC):
                nc.tensor.matmul(
                    ppr,
                    lhsT=attnT[:, c * S + s:c * S + s + 1],
                    rhs=xnat[:, s, c, :],
                    start=(c == 0),
                    stop=(c == CC - 1),
                )
            orow = row_p.tile([1, H], F32, name="orow")
            nc.scalar.activation(orow, ppr, Tanh)
            nc.gpsimd.dma_start(out_h.ap()[s:s + 1], orow)

    nc.finalize()
    return nc


_CACHE = {}


def _get_nc():
    if "nc" not in _CACHE:
        _CACHE["nc"] = build_program()
    return _CACHE["nc"]


def make_in_maps(code_feat, node_mask, W_sa, b_sa, w_sc):
    ident = np.eye(128, dtype=np.float32)
    x16 = np.asarray(code_feat, dtype=np.float16)
    w16 = np.asarray(W_sa, dtype=np.float16)
    wsc16 = np.asarray(w_sc, dtype=np.float16)
    in_maps = []
    for i in range(NCORES):
        sl = slice(i * S, (i + 1) * S)
        in_maps.append({
            "x": np.ascontiguousarray(x16[sl]),
            "xT": np.ascontiguousarray(x16[sl].transpose(0, 2, 1)),
            "mask": np.ascontiguousarray(node_mask[sl], dtype=np.int32),
            "w_sa": w16,
            "b_sa": np.asarray(b_sa, dtype=np.float32),
            "w_sc": wsc16,
            "ident": ident,
        })
    return in_maps


def kernel(code_feat, node_mask, W_sa, b_sa, w_sc, b_sc=None, **_ignored):
    code_feat = np.asarray(code_feat)
    node_mask = np.asarray(node_mask)
    nc = _get_nc()
    in_maps = make_in_maps(code_feat, node_mask, W_sa, b_sa, w_sc)
    res = run_bass_kernel_spmd(nc, in_maps, list(range(NCORES)))
    out = np.concatenate([r["out"] for r in res.results], axis=0)
    return out.astype(np.float32)


# revision 23
# speedup vs baseline: 1.5227x; 1.0211x over previous
"""Trainium2 Bass kernel for nn_CFGEmbeder (masked attention pooling).

Reference computation (per batch sample, B=128, N=512 nodes, H=512):
    h      = tanh(code_feat @ W_sa + b_sa)         [N, H]
    scores = h @ w_sc (+ b_sc)                      [N]
    attn   = softmax(scores masked by node_mask)    [N]
    out    = tanh(attn @ code_feat)                 [H]

Sharding: pure data parallel over batch; 16 samples per NeuronCore x 8 cores.
b_sc is dropped: softmax is shift invariant, so it cannot affect the output.

Per-core device algorithm (matmuls in fp16 with fp32 PSUM accumulation):
  - x, W_sa, w_sc are cast to fp16 host-side; the host also supplies x
    pre-transposed (xT, partition=feature) so no on-device transpose is
    needed. xT loads ride the otherwise-idle SWDGE queue (feeds matmul1
    immediately); x-natural loads ride the ACT HWDGE ring (only needed by
    the late pooling phase, so ScalarE's busy stream can delay them freely).
  - mm1: hT[m] = sum_k W[k,m].T @ xT[k]  -> PSUM; tanh+bias fused on ScalarE.
  - scores_s [1, 512] = sum_m w_sc[m-chunk].T @ tanh_hT[m]: M=1 row matmuls
    (1-column weight loads — cheap; the N=1 column formulation costs a full
    128-col LDWEIGHTS per matmul and dominated the first profile).
  - score rows go SBUF -> DRAM scratch -> one gathered [16, 512] load
    (engines cannot move data across partitions; an SBUF->SBUF DMA would
    race the xbar transposes - a known HW deadlock - so bounce via DRAM).
  - one batched masked softmax: masked = (scores + 1000) * mask (the shift
    keeps softmax exact), exp with accum_out giving row sums in one ACT op.
  - attn -> attnT via 4 PE transposes (identity moving operand).
  - pooled_s [1, 512] = sum_c attnT[c,s].T @ x_nat[s,c]: M=1 row matmuls.
  - per-sample fused tanh on ScalarE, row stored straight to the output.
"""

from contextlib import ExitStack

import numpy as np

import concourse.bass as bass
import concourse.bacc as bacc
import concourse.mybir as mybir
import concourse.tile as tile
from concourse.bass_utils import run_bass_kernel_spmd

F16 = mybir.dt.float16
F32 = mybir.dt.float32
I32 = mybir.dt.int32

B, N, H = 128, 512, 512
NCORES = 8
S = B // NCORES          # samples per core
KC = H // 128            # 4 contraction chunks
MC = H // 128            # 4 output-feature chunks
CC = N // 128            # 4 node chunks
GS = 4                   # samples per load/transpose DMA group
NG = S // GS
MASK_SHIFT = 1000.0      # (scores + SHIFT) * mask; softmax is shift invariant


def build_program():
    nc = bacc.Bacc(trn_type="TRN2", target_bir_lowering=False,
                   num_devices=NCORES)

    x_h = nc.dram_tensor("x", [S, N, H], F16, kind="ExternalInput")
    xt_h = nc.dram_tensor("xT", [S, H, N], F16, kind="ExternalInput")
    mask_h = nc.dram_tensor("mask", [S, N], I32, kind="ExternalInput")
    wsa_h = nc.dram_tensor("w_sa", [H, H], F16, kind="ExternalInput")
    bsa_h = nc.dram_tensor("b_sa", [H], F32, kind="ExternalInput")
    wsc_h = nc.dram_tensor("w_sc", [H], F16, kind="ExternalInput")
    id_h = nc.dram_tensor("ident", [128, 128], F32, kind="ExternalInput")
    out_h = nc.dram_tensor("out", [S, H], F32, kind="ExternalOutput")
    sc_h = nc.dram_tensor("score_scratch", [S, N], F32)

    x = x_h.ap()
    Tanh = mybir.ActivationFunctionType.Tanh
    Exp = mybir.ActivationFunctionType.Exp
    Alu = mybir.AluOpType

    with tile.TileContext(nc) as tc, ExitStack() as ctx:
        const = ctx.enter_context(tc.tile_pool(name="const", bufs=1))
        xnat_p = ctx.enter_context(tc.tile_pool(name="xnat", bufs=1))
        xt_p = ctx.enter_context(tc.tile_pool(name="xt", bufs=1))
        th_p = ctx.enter_context(tc.tile_pool(name="th", bufs=2))
        row_p = ctx.enter_context(tc.tile_pool(name="row", bufs=3))
        sm_p = ctx.enter_context(tc.tile_pool(name="sm", bufs=1))
        ph_p = ctx.enter_context(tc.tile_pool(name="ph", bufs=4, space="PSUM"))
        pr_p = ctx.enter_context(tc.tile_pool(name="pr", bufs=3, space="PSUM"))
        pa_p = ctx.enter_context(tc.tile_pool(name="pa", bufs=1, space="PSUM"))

        # ---- constants (ACT HWDGE ring; SWDGE would serialize the xbar) ----
        Wf = const.tile([128, KC, H], F16, name="Wf")
        nc.scalar.dma_start(Wf, wsa_h.ap().rearrange("(k p) h -> p k h", p=128))
        wsc = const.tile([128, MC], F16, name="wsc")
        nc.scalar.dma_start(wsc, wsc_h.ap().rearrange("(c p) -> p c", p=128))
        bsa = const.tile([128, MC], F32, name="bsa")
        nc.scalar.dma_start(bsa, bsa_h.ap().rearrange("(c p) -> p c", p=128))
        idf = const.tile([128, 128], F32, name="idf")
        nc.scalar.dma_start(idf, id_h.ap())
        maski = const.tile([S, N], I32, name="maski")
        nc.scalar.dma_start(maski, mask_h.ap())
        maskf = const.tile([S, N], F32, name="maskf")
        nc.vector.tensor_copy(maskf, maski)

        # ---- x loads: all on the SWDGE queue (gpsimd issues nothing else,
        # so DMA issues never block a compute engine's instruction stream;
        # an HWDGE ring issue stalls its engine until the ring drains).
        # xT groups first (feed matmul1 right away), x-natural after (only
        # the late pooling phase needs it).
        xnat = xnat_p.tile([128, S, CC, H], F16, name="xnat")
        # xt layout: [128(u=feat%128), s, k(feat chunk), n(node)]
        xt = xt_p.tile([128, S, KC, N], F16, name="xt")
        xth = xt_h.ap()
        for g in range(S // 2):
            sl = slice(g * 2, (g + 1) * 2)
            nc.gpsimd.dma_start(
                xt[:, sl], xth[sl].rearrange("s (k p) n -> p s k n", p=128))
        for g in range(NG):
            sl = slice(g * GS, (g + 1) * GS)
            nc.gpsimd.dma_start(
                xnat[:, sl], x[sl].rearrange("s (c p) h -> p s c h", p=128))

        # ---- phase A: paired-sample matmul1 + tanh + scores rows ----
        # Samples are processed in pairs sharing each W stationary load:
        # LDWEIGHTS(k,m) then two matmuls streaming both samples' xT.
        for s0 in range(0, S, 2):
            ths = [th_p.tile([128, MC, N], F16, name=f"th{i}") for i in (0, 1)]
            for m in range(MC):
                phs = [ph_p.tile([128, N], F32, name="ph") for _ in (0, 1)]
                for k in range(KC):
                    for i in (0, 1):
                        nc.tensor.matmul(
                            phs[i],
                            lhsT=Wf[:, k, m * 128:(m + 1) * 128],
                            rhs=xt[:, s0 + i, k, :],
                            start=(k == 0),
                            stop=(k == KC - 1),
                        )
                for i in (0, 1):
                    nc.scalar.activation(ths[i][:, m, :], phs[i], Tanh,
                                         bias=bsa[:, m:m + 1])

            for i in (0, 1):
                psr = pr_p.tile([1, N], F32, name="prow")
                for m in range(MC):
                    nc.tensor.matmul(
                        psr,
                        lhsT=wsc[:, m:m + 1],
                        rhs=ths[i][:, m, :],
                        start=(m == 0),
                        stop=(m == MC - 1),
                    )
                srow = row_p.tile([1, N], F32, name="srow")
                nc.vector.tensor_copy(srow, psr)
                nc.gpsimd.dma_start(sc_h.ap()[s0 + i:s0 + i + 1], srow)

        # ---- phase B: softmax over nodes for all samples at once ----
        # (bounce through DRAM: engines cannot move data across partitions)
        scores = sm_p.tile([S, N], F32, name="scores")
        nc.gpsimd.dma_start(scores, sc_h.ap())

        masked = sm_p.tile([S, N], F32, name="masked")
        nc.vector.scalar_tensor_tensor(masked, scores, MASK_SHIFT, maskf,
                                       op0=Alu.add, op1=Alu.mult)
        nmax = sm_p.tile([S, 1], F32, name="nmax")
        nc.vector.tensor_reduce(nmax, masked, axis=mybir.AxisListType.X,
                                op=Alu.max, negate=True)
        ex = sm_p.tile([S, N], F32, name="ex")
        esum = sm_p.tile([S, 1], F32, name="esum")
        nc.scalar.activation(ex, masked, Exp, bias=nmax, accum_out=esum)
        rinv = sm_p.tile([S, 1], F32, name="rinv")
        nc.vector.reciprocal(rinv, esum)
        attn = sm_p.tile([S, N], F32, name="attn")
        nc.vector.tensor_scalar_mul(attn, ex, rinv)

        # Keep the PE HAM window busy through the softmax bubble so the
        # pooling matmuls run at 2.4 GHz instead of re-warming from 1.2.
        phw = ph_p.tile([128, N], F32, name="ph")
        for w in range(20):
            nc.tensor.matmul(phw, lhsT=Wf[:, 0, 0:128], rhs=xt[:, S - 1, 0, :],
                             start=(w == 0), stop=(w == 19))

        psum_aT = pa_p.tile([128, CC * S], F32, name="paT")
        for c in range(CC):
            nc.tensor.transpose(psum_aT[:, c * S:(c + 1) * S],
                                attn[:, c * 128:(c + 1) * 128],
                                idf[0:S, 0:S])
        attnT = sm_p.tile([128, CC * S], F16, name="attnT")
        nc.vector.tensor_copy(attnT, psum_aT)

        # ---- phase C: attention pooling ----
        for s in range(S):
            ppr = pr_p.tile([1, H], F32, name="prow")
            for c in range(CC):
                nc.tensor.matmul(
                    ppr,
                    lhsT=attnT[:, c * S + s:c * S + s + 1],
                    rhs=xnat[:, s, c, :],
                    start=(c == 0),
                    stop=(c == CC - 1),
                )
            orow = row_p.tile([1, H], F32, name="orow")
            nc.scalar.activation(orow, ppr, Tanh)
            nc.gpsimd.dma_start(out_h.ap()[s:s + 1], orow)

    nc.finalize()
    return nc


_CACHE = {}


def _get_nc():
    if "nc" not in _CACHE:
        _CACHE["nc"] = build_program()
    return _CACHE["nc"]


def make_in_maps(code_feat, node_mask, W_sa, b_sa, w_sc):
    ident = np.eye(128, dtype=np.float32)
    x16 = np.asarray(code_feat, dtype=np.float16)
    w16 = np.asarray(W_sa, dtype=np.float16)
    wsc16 = np.asarray(w_sc, dtype=np.float16)
    in_maps = []
    for i in range(NCORES):
        sl = slice(i * S, (i + 1) * S)
        in_maps.append({
            "x": np.ascontiguousarray(x16[sl]),
            "xT": np.ascontiguousarray(x16[sl].transpose(0, 2, 1)),
            "mask": np.ascontiguousarray(node_mask[sl], dtype=np.int32),
            "w_sa": w16,
            "b_sa": np.asarray(b_sa, dtype=np.float32),
            "w_sc": wsc16,
            "ident": ident,
        })
    return in_maps


def kernel(code_feat, node_mask, W_sa, b_sa, w_sc, b_sc=None, **_ignored):
    code_feat = np.asarray(code_feat)
    node_mask = np.asarray(node_mask)
    nc = _get_nc()
    in_maps = make_in_maps(code_feat, node_mask, W_sa, b_sa, w_sc)
    res = run_bass_kernel_spmd(nc, in_maps, list(range(NCORES)))
    out = np.concatenate([r["out"] for r in res.results], axis=0)
    return out.astype(np.float32)
